# revision 1
# baseline (speedup 1.0000x reference)
"""Trainium2 Bass kernel: fused concat-linear attention map + softmax.

reference:  scores[b,h,n] = key[b,n,:]@Wk[h,:] + query[b,0,:]@Wq[h,:] + bias[h]
            attn = softmax over n              (B=16, N=20000, D=256, H=8)

Sharding: batch dim B=16 split across 8 cores (2 batches/core), weights
replicated.  Per batch the kernel streams key (20.5 MB f32) through:
  DMA (2 MB chunks, natural layout)
    -> PE transpose of 128x128 blocks (exact data movement; gives d on
       partitions, which the matmul contraction requires)
    -> DVE/ACT copy PSUM->SBUF
    -> PE matmul vs tiny stationary WkT [128,8] (float32r moving rate:
       1 cycle/row) accumulating the two d-halves in PSUM
    -> ScalarE fused exp(x + (qWq+b)[h]) with per-chunk accumulated sums
       (softmax without max-subtraction: scores are O(+-7) so f32 exp is
       safe and mathematically identical)
    -> DVE scale by 1/sum, contiguous DMA out.
"""

import sys

import numpy as np

for _p in ("/opt/trn_rl_repo",):
    if _p not in sys.path:
        sys.path.append(_p)

from contextlib import ExitStack

import concourse.bass as bass
import concourse.bacc as bacc
import concourse.tile as tile
from concourse import mybir
from concourse.masks import make_identity

B, N, D, H = 16, 20000, 256, 8
NCORES = 8
BPC = B // NCORES  # batches per core
P = 128
CHUNK = 512  # n-columns per score chunk (= one PSUM bank of f32)
LOAD_SUB = 16  # 128-row subtiles per load DMA (2048 rows = 2 MB)
F32 = mybir.dt.float32
F32R = mybir.dt.float32r


def _ceil_div(a, b):
    return (a + b - 1) // b


def build_kernel(n=N, bpc=BPC, score_dtype=F32R, tp_dtype=F32):
    nc = bacc.Bacc("TRN2", target_bir_lowering=False, debug=False)
    q_in = nc.declare_dram_parameter("q", [bpc, D], F32, isOutput=False)
    k_in = nc.declare_dram_parameter("k", [bpc, n, D], F32, isOutput=False)
    w_in = nc.declare_dram_parameter("w", [H, 2 * D], F32, isOutput=False)
    b_in = nc.declare_dram_parameter("b", [H], F32, isOutput=False)
    out = nc.declare_dram_parameter("out", [bpc, H, n], F32, isOutput=True)

    nchunks = _ceil_div(n, CHUNK)

    with ExitStack() as ctx:
        tc = ctx.enter_context(tile.TileContext(nc))
        consts = ctx.enter_context(tc.tile_pool(name="consts", bufs=1))
        loads = ctx.enter_context(tc.tile_pool(name="loads", bufs=3))
        kts = ctx.enter_context(tc.tile_pool(name="kts", bufs=3))
        probp = ctx.enter_context(tc.tile_pool(name="prob", bufs=1))
        small = ctx.enter_context(tc.tile_pool(name="small", bufs=2))
        psum_kt = ctx.enter_context(tc.tile_pool(name="psum_kt", bufs=2, space="PSUM"))
        psum_sc = ctx.enter_context(tc.tile_pool(name="psum_sc", bufs=2, space="PSUM"))
        psum_mi = ctx.enter_context(tc.tile_pool(name="psum_mi", bufs=1, space="PSUM"))

        identity = consts.tile([P, P], F32)
        make_identity(nc, identity)
        id_t = identity[:, :] if tp_dtype == F32 else identity[:, :].bitcast(tp_dtype)

        # --- constants: W transposed to [d, h] chunks, bias, queries -------
        w_sb = consts.tile([H, 2 * D], F32)
        nc.sync.dma_start(out=w_sb[:, :], in_=w_in[:, :])
        b_sb = consts.tile([H, 1], F32)
        nc.sync.dma_start(out=b_sb[:, :], in_=b_in[:])

        # wqT[:, c, :]: WqT halves (exact f32); wkT[:, c, :]: WkT halves,
        # rounded to the score matmul dtype during the PSUM->SBUF copy.
        wqT = consts.tile([P, 2, H], F32)
        wkT = consts.tile([P, 2, H], score_dtype)
        for c in range(4):
            pt = psum_mi.tile([P, H], F32, tag="mi")
            nc.tensor.transpose(pt[:, :], w_sb[:, c * P:(c + 1) * P], identity[:H, :H])
            dst = wqT[:, c, :] if c < 2 else wkT[:, c - 2, :]
            nc.vector.tensor_copy(out=dst, in_=pt[:, :])

        q_sb = consts.tile([1, bpc, D], F32)
        nc.sync.dma_start(out=q_sb[:, :, :], in_=q_in[:, :])
        qT = consts.tile([P, bpc, 2], F32)
        for i in range(bpc):
            for c in range(2):
                pt = psum_mi.tile([P, 1], F32, tag="mi")
                nc.tensor.transpose(
                    pt[:, :], q_sb[0:1, i, c * P:(c + 1) * P], identity[:1, :1]
                )
                nc.vector.tensor_copy(out=qT[:, i, c:c + 1], in_=pt[:, :])

        # qb[:, i] = Wq @ q_i + b   (full-f32 matmul; 1-row stream, trivial)
        qb = consts.tile([H, bpc], F32)
        for i in range(bpc):
            qp = psum_mi.tile([H, 1], F32, tag="mi")
            nc.tensor.matmul(
                qp[:, :], wqT[:, 0, :], qT[:, i, 0:1], start=True, stop=False
            )
            nc.tensor.matmul(
                qp[:, :], wqT[:, 1, :], qT[:, i, 1:2], start=False, stop=True
            )
            nc.vector.tensor_add(qb[:, i:i + 1], qp[:, :], b_sb[:, :])

        wk0 = wkT[:, 0, :]
        wk1 = wkT[:, 1, :]

        # --- main loop ------------------------------------------------------
        load_rows = LOAD_SUB * P
        nloads = _ceil_div(n, load_rows)
        for i in range(bpc):
            prob = probp.tile([H, n], F32, tag="prob")
            sums = small.tile([H, nchunks], F32, tag="sums")
            gchunk = 0
            for L in range(nloads):
                n0 = L * load_rows
                rows = min(load_rows, n - n0)
                full_sub = rows // P
                rem = rows - full_sub * P
                ld = loads.tile([P, LOAD_SUB, D], F32, tag="load")
                if full_sub:
                    nc.sync.dma_start(
                        out=ld[:, :full_sub, :],
                        in_=k_in[i, n0:n0 + full_sub * P, :].rearrange(
                            "(s p) d -> p s d", p=P
                        ),
                    )
                if rem:
                    nc.sync.dma_start(
                        out=ld[:rem, full_sub, :],
                        in_=k_in[i, n0 + full_sub * P:n0 + rows, :],
                    )
                for sc in range(_ceil_div(rows, CHUNK)):
                    w = min(CHUNK, rows - sc * CHUNK)
                    nsub = _ceil_div(w, P)
                    kt0 = psum_kt.tile([P, CHUNK], F32, tag="kt0")
                    kt1 = psum_kt.tile([P, CHUNK], F32, tag="kt1")
                    def _tp(ap):
                        return ap if tp_dtype == F32 else ap.bitcast(tp_dtype)

                    for t in range(nsub):
                        tw = min(P, w - t * P)
                        s = sc * (CHUNK // P) + t
                        nc.tensor.transpose(
                            _tp(kt0[:, t * P:t * P + tw]),
                            _tp(ld[:tw, s, 0:P]),
                            id_t[:tw, :tw],
                        )
                        nc.tensor.transpose(
                            _tp(kt1[:, t * P:t * P + tw]),
                            _tp(ld[:tw, s, P:2 * P]),
                            id_t[:tw, :tw],
                        )
                    k0 = kts.tile([P, CHUNK], score_dtype, tag="k0")
                    k1 = kts.tile([P, CHUNK], score_dtype, tag="k1")
                    # alternate engines so PSUM->SBUF copy load is split
                    def _copy_v(o, s):
                        nc.vector.tensor_copy(out=o, in_=s)

                    def _copy_a(o, s):
                        nc.scalar.copy(out=o, in_=s)

                    eng_a = _copy_v if gchunk % 2 == 0 else _copy_a
                    eng_b = _copy_a if gchunk % 2 == 0 else _copy_v
                    eng_a(k0[:, :w], kt0[:, :w])
                    eng_b(k1[:, :w], kt1[:, :w])
                    scp = psum_sc.tile([H, CHUNK], F32, tag="sc")
                    nc.tensor.matmul(
                        scp[:, :w], wk0, k0[:, :w], start=True, stop=False
                    )
                    nc.tensor.matmul(
                        scp[:, :w], wk1, k1[:, :w], start=False, stop=True
                    )
                    nc.scalar.activation(
                        out=prob[:, n0 + sc * CHUNK:n0 + sc * CHUNK + w],
                        in_=scp[:, :w],
                        func=mybir.ActivationFunctionType.Exp,
                        bias=qb[:, i:i + 1],
                        scale=1.0,
                        accum_out=sums[:, gchunk:gchunk + 1],
                    )
                    gchunk += 1
            assert gchunk == nchunks
            tot = small.tile([H, 1], F32, tag="tot")
            nc.vector.reduce_sum(out=tot[:, :], in_=sums[:, :], axis=mybir.AxisListType.X)
            rec = small.tile([H, 1], F32, tag="rec")
            nc.vector.reciprocal(out=rec[:, :], in_=tot[:, :])
            nc.vector.tensor_scalar_mul(prob[:, :], prob[:, :], rec[:, :])
            nc.sync.dma_start(out=out[i, :, :], in_=prob[:, :])

    nc.compile()
    return nc


_NC_CACHE = {}


def _get_nc():
    if "nc" not in _NC_CACHE:
        _NC_CACHE["nc"] = build_kernel()
    return _NC_CACHE["nc"]


def kernel(query, key, W, b):
    from concourse.bass_utils import run_bass_kernel_spmd

    query = np.ascontiguousarray(np.asarray(query, np.float32).reshape(B, D))
    key = np.ascontiguousarray(np.asarray(key, np.float32))
    W = np.ascontiguousarray(np.asarray(W, np.float32))
    b = np.ascontiguousarray(np.asarray(b, np.float32))

    nc = _get_nc()
    in_maps = []
    for c in range(NCORES):
        s = slice(BPC * c, BPC * (c + 1))
        in_maps.append(
            {
                "q": query[s],
                "k": key[s],
                "w": W,
                "b": b,
            }
        )
    res = run_bass_kernel_spmd(nc, in_maps, list(range(NCORES))).results
    return np.concatenate([res[c]["out"] for c in range(NCORES)], axis=0)



# revision 19
# speedup vs baseline: 1.4048x; 1.4048x over previous
"""Trainium2 Bass kernel: fused concat-linear attention map + softmax.

reference:  scores[b,h,n] = key[b,n,:]@Wk[h,:] + query[b,0,:]@Wq[h,:] + bias[h]
            attn = softmax over n              (B=16, N=20000, D=256, H=8)

Sharding: batch dim B=16 split across 8 cores (2 batches/core), weights
replicated.

Per-core design (v2): avoid the PE-transpose-per-128x128-block pipeline
(which saturated the tensor engine at ~300us) by
  1. p-major loads: ld[p, s, d] = key[n0 + p*S + s, d] -> each of the 128
     partition lines is S*1KB contiguous in HBM (16KB DMA descriptors,
     near-line-rate ~358 GB/s).
  2. One DVE stream-transpose per load ([128, S*256], 32x32 blocks):
     ldt[32a+v, (s,b,u)] = key[n0+(32a+u)*S+s, 32b+v] -- d%32 moves onto
     partitions.
  3. 8 accumulating f32r matmuls with block-diagonal delta-weights
     lhsT_b[(a,v), 8a'+h] = (a==a') * Wk[h, 32b+v] compute scores for all
     128*S rows into ONE [32, S*32] PSUM bank (output partition = 8a'+h).
  4. One ScalarE exp per load (bias = q-term+b), writing prob with a
     strided AP so each partition holds a contiguous n-run; accum_out
     gives per-load softmax partial sums.
  5. Softmax normalization via tiny reduction matmuls + ACT copy-scale;
     contiguous output DMA per a'-group.
"""

import sys

import numpy as np

for _p in ("/opt/trn_rl_repo",):
    if _p not in sys.path:
        sys.path.append(_p)

from contextlib import ExitStack

import concourse.bass as bass
import concourse.bacc as bacc
import concourse.tile as tile
from concourse import mybir
from concourse.masks import make_identity

B, N, D, H = 16, 20000, 256, 8
NCORES = 8
BPC = B // NCORES  # batches per core
P = 128
F32 = mybir.dt.float32
F32R = mybir.dt.float32r
BF16 = mybir.dt.bfloat16

# p-major load plan: 9 loads of [128, 16] rows + 1 of [128, 12] + 32-row tail
LOADS = [16] * 9 + [12]  # S per load; rows = 128*S
MAIN_ROWS = sum(128 * s for s in LOADS)  # 19968
TAIL_ROWS = N - MAIN_ROWS  # 32


def _r(ap):
    return ap.bitcast(F32R)


def build_kernel(n=N, bpc=BPC):
    nc = bacc.Bacc("TRN2", target_bir_lowering=False, debug=False)
    q_in = nc.declare_dram_parameter("q", [bpc, D], F32, isOutput=False)
    k_in = nc.declare_dram_parameter("k", [bpc, n, D], F32, isOutput=False)
    w_in = nc.declare_dram_parameter("w", [H, 2 * D], F32, isOutput=False)
    b_in = nc.declare_dram_parameter("b", [H], F32, isOutput=False)
    out = nc.declare_dram_parameter("out", [bpc, H, n], F32, isOutput=True)

    ncols_main = sum(32 * s for s in LOADS)  # per-partition prob cols (4992)

    with ExitStack() as ctx:
        tc = ctx.enter_context(tile.TileContext(nc))
        consts = ctx.enter_context(tc.tile_pool(name="consts", bufs=1))
        loads = ctx.enter_context(tc.tile_pool(name="loads", bufs=3))
        ldts = ctx.enter_context(tc.tile_pool(name="ldts", bufs=3))
        probp = ctx.enter_context(tc.tile_pool(name="prob", bufs=2))
        small = ctx.enter_context(tc.tile_pool(name="small", bufs=2))
        psum_sc = ctx.enter_context(tc.tile_pool(name="psum_sc", bufs=3, space="PSUM"))
        psum_mi = ctx.enter_context(tc.tile_pool(name="psum_mi", bufs=2, space="PSUM"))

        identity = consts.tile([P, P], F32)
        make_identity(nc, identity)

        # --- constants ------------------------------------------------------
        w_sb = consts.tile([H, 2 * D], F32)
        nc.sync.dma_start(out=w_sb[:, :], in_=w_in[:, :])
        b_sb = consts.tile([H, 1], F32)
        nc.sync.dma_start(out=b_sb[:, :], in_=b_in[:])
        q_sb = consts.tile([1, bpc, D], F32)
        nc.sync.dma_start(out=q_sb[:, :, :], in_=q_in[:, :])

        # wqT[:, c, :]: Wq halves transposed to [d, h]
        wqT = consts.tile([P, 2, H], F32)
        for c in range(2):
            pt = psum_mi.tile([P, H], F32, tag="mi")
            nc.tensor.transpose(pt[:, :], w_sb[:, c * P:(c + 1) * P], identity[:H, :H])
            nc.vector.tensor_copy(out=wqT[:, c, :], in_=pt[:, :])

        # delta[(a,v), b, 8a+h] = Wk[h, 32b+v]; wkT_small[v, b, h] = Wk[h, 32b+v]
        delta32 = consts.tile([P, 8, 32], F32)
        nc.vector.memset(delta32[:, :, :], 0.0)
        wkT_small = consts.tile([32, 8, H], BF16)
        for bb in range(8):
            pt = psum_mi.tile([32, H], F32, tag="mi")
            nc.tensor.transpose(
                pt[:, :], w_sb[:, D + 32 * bb:D + 32 * (bb + 1)], identity[:H, :H]
            )
            nc.vector.tensor_copy(out=wkT_small[:, bb, :], in_=pt[:, :])
            for a in range(4):
                nc.vector.tensor_copy(
                    out=delta32[32 * a:32 * (a + 1), bb, 8 * a:8 * (a + 1)],
                    in_=pt[:, :],
                )
        delta = consts.tile([P, 8, 32], BF16)
        nc.vector.tensor_copy(out=delta[:, :, :], in_=delta32[:, :, :])

        # E[h', 8a+h] = (h'==h): [I8 I8 I8 I8]  (f32: feeds tiny f32 matmuls)
        E = consts.tile([H, 32], F32)
        for a in range(4):
            nc.vector.tensor_copy(out=E[:, 8 * a:8 * (a + 1)], in_=identity[:H, :H])
        # F[(8a+h), h'] = (h==h'): 4 stacked I8 blocks, via transpose of E
        F = consts.tile([32, H], F32)
        ptF = psum_mi.tile([32, H], F32, tag="mi")
        nc.tensor.transpose(ptF[:, :], E[:, :], identity[:H, :H])
        nc.vector.tensor_copy(out=F[:, :], in_=ptF[:, :])

        # qT: query transposed to [d, i, c]
        qT = consts.tile([P, bpc, 2], F32)
        for i in range(bpc):
            for c in range(2):
                pt = psum_mi.tile([P, 1], F32, tag="mi")
                nc.tensor.transpose(
                    pt[:, :], q_sb[0:1, i, c * P:(c + 1) * P], identity[:1, :1]
                )
                nc.vector.tensor_copy(out=qT[:, i, c:c + 1], in_=pt[:, :])

        # qb8[h, i] = Wq @ q_i + bias
        qb8 = consts.tile([H, bpc], F32)
        for i in range(bpc):
            qp = psum_mi.tile([H, 1], F32, tag="mi")
            nc.tensor.matmul(
                qp[:, :], wqT[:, 0, :], qT[:, i, 0:1], start=True, stop=False
            )
            nc.tensor.matmul(
                qp[:, :], wqT[:, 1, :], qT[:, i, 1:2], start=False, stop=True
            )
            nc.vector.tensor_add(qb8[:, i:i + 1], qp[:, :], b_sb[:, :])
        # qb32[8a+h, i] = qb8[h, i]
        qb32 = consts.tile([32, bpc], F32)
        qp32 = psum_mi.tile([32, bpc], F32, tag="mi")
        nc.tensor.matmul(qp32[:, :], E[:, :], qb8[:, :], start=True, stop=True)
        nc.vector.tensor_copy(out=qb32[:, :], in_=qp32[:, :])

        # --- main loop ------------------------------------------------------
        for i in range(bpc):
            prob = probp.tile([32, ncols_main], F32, tag="prob")
            prob_t = probp.tile([H, TAIL_ROWS], F32, tag="probt")
            sums = small.tile([32, len(LOADS)], F32, tag="sums")
            sums_t = small.tile([H, 1], F32, tag="sumst")

            n0 = 0
            c0 = 0
            for L, S in enumerate(LOADS):
                rows = 128 * S
                ld = loads.tile([P, S, D], BF16, tag="load")
                nc.gpsimd.dma_start(
                    out=ld[:, :, :],
                    in_=k_in[i, n0:n0 + rows, :].rearrange("(p s) d -> p s d", s=S),
                )
                ldt = ldts.tile([P, S, 8, 32], BF16, tag="ldt")
                nc.vector.transpose(
                    out=ldt[:, :, :, :].rearrange("p s b u -> p (s b u)"),
                    in_=ld[:, :, :].rearrange("p s d -> p (s d)"),
                )
                scp = psum_sc.tile([32, S, 32], F32, tag="sc")
                for bb in range(8):
                    nc.tensor.matmul(
                        scp[:, :, :],
                        delta[:, bb, :],
                        ldt[:, :, bb, :],
                        start=(bb == 0),
                        stop=(bb == 7),
                    )
                # prob cols: c0 + u*S + s  (natural n order per partition)
                nc.scalar.activation(
                    out=prob[:, c0:c0 + 32 * S].rearrange("p (u s) -> p s u", s=S),
                    in_=scp[:, :, :],
                    func=mybir.ActivationFunctionType.Exp,
                    bias=qb32[:, i:i + 1],
                    scale=1.0,
                    accum_out=sums[:, L:L + 1],
                )
                n0 += rows
                c0 += 32 * S

            # 32-row tail: natural layout, K=32 matmuls on partitions 0-7
            ld_tb = loads.tile([TAIL_ROWS, D], BF16, tag="loadtb")
            nc.gpsimd.dma_start(out=ld_tb[:, :], in_=k_in[i, MAIN_ROWS:n, :])
            ldt_tb = ldts.tile([TAIL_ROWS, 8, 32], BF16, tag="ldttb")
            nc.vector.transpose(
                out=ldt_tb[:, :, :].rearrange("p b u -> p (b u)"),
                in_=ld_tb[:, :],
            )
            sct = psum_sc.tile([H, TAIL_ROWS], F32, tag="sct")
            for bb in range(8):
                nc.tensor.matmul(
                    sct[:, :],
                    wkT_small[:, bb, :],
                    ldt_tb[:, bb, :],
                    start=(bb == 0),
                    stop=(bb == 7),
                )
            nc.scalar.activation(
                out=prob_t[:, :],
                in_=sct[:, :],
                func=mybir.ActivationFunctionType.Exp,
                bias=qb8[:, i:i + 1],
                scale=1.0,
                accum_out=sums_t[:, :],
            )

            # totals: tot8[h] = sum_a sums[(8a+h), :] + sums_t[h]  (all-f32 tinies)
            sums_r = small.tile([32, 1], F32, tag="sumsr")
            nc.vector.reduce_sum(
                out=sums_r[:, :], in_=sums[:, :], axis=mybir.AxisListType.X
            )
            tot8 = psum_mi.tile([H, 1], F32, tag="mi")
            nc.tensor.matmul(
                tot8[:, :], F[:, :], sums_r[:, :], start=True, stop=False
            )
            nc.tensor.matmul(
                tot8[:, :], identity[:H, :H], sums_t[:, :], start=False, stop=True
            )
            rec8 = small.tile([H, 1], F32, tag="rec8")
            nc.vector.reciprocal(out=rec8[:, :], in_=tot8[:, :])
            rec32p = psum_mi.tile([32, 1], F32, tag="mi")
            nc.tensor.matmul(
                rec32p[:, :], E[:, :], rec8[:, :], start=True, stop=True
            )
            rec32 = small.tile([32, 1], F32, tag="rec32")
            nc.vector.tensor_copy(out=rec32[:, :], in_=rec32p[:, :])

            # normalize
            nc.scalar.activation(
                out=prob[:, :],
                in_=prob[:, :],
                func=mybir.ActivationFunctionType.Copy,
                bias=0.0,
                scale=rec32[:, :],
            )
            nc.scalar.activation(
                out=prob_t[:, :],
                in_=prob_t[:, :],
                func=mybir.ActivationFunctionType.Copy,
                bias=0.0,
                scale=rec8[:, :],
            )

            # output: per a'-group, n = n_base(L) + a*32*S + (u*S+s)
            full = [(L, S) for L, S in enumerate(LOADS) if S == 16]
            nfull = len(full)  # 9
            for a in range(4):
                # loads 0..8 (S=16): n = L*2048 + a*512 + c
                nc.sync.dma_start(
                    out=out[i, :, 0:nfull * 2048].rearrange(
                        "h (l x) -> h l x", x=2048
                    )[:, :, a * 512:(a + 1) * 512],
                    in_=prob[8 * a:8 * (a + 1), 0:nfull * 512].rearrange(
                        "h (l c) -> h l c", c=512
                    ),
                )
                # load 9 (S=12): n = 18432 + a*384 + c
                nc.sync.dma_start(
                    out=out[i, :, nfull * 2048 + a * 384:nfull * 2048 + (a + 1) * 384],
                    in_=prob[8 * a:8 * (a + 1), nfull * 512:nfull * 512 + 384],
                )
            nc.sync.dma_start(out=out[i, :, MAIN_ROWS:n], in_=prob_t[:, :])

    nc.compile()
    return nc


_NC_CACHE = {}


def _get_nc():
    if "nc" not in _NC_CACHE:
        _NC_CACHE["nc"] = build_kernel()
    return _NC_CACHE["nc"]


def kernel(query, key, W, b):
    from concourse.bass_utils import run_bass_kernel_spmd

    query = np.ascontiguousarray(np.asarray(query, np.float32).reshape(B, D))
    key = np.ascontiguousarray(np.asarray(key, np.float32))
    W = np.ascontiguousarray(np.asarray(W, np.float32))
    b = np.ascontiguousarray(np.asarray(b, np.float32))

    nc = _get_nc()
    in_maps = []
    for c in range(NCORES):
        s = slice(BPC * c, BPC * (c + 1))
        in_maps.append(
            {
                "q": query[s],
                "k": key[s],
                "w": W,
                "b": b,
            }
        )
    res = run_bass_kernel_spmd(nc, in_maps, list(range(NCORES))).results
    return np.concatenate([res[c]["out"] for c in range(NCORES)], axis=0)


# revision 20
# speedup vs baseline: 1.4091x; 1.0030x over previous
"""Trainium2 Bass kernel: fused concat-linear attention map + softmax.

reference:  scores[b,h,n] = key[b,n,:]@Wk[h,:] + query[b,0,:]@Wq[h,:] + bias[h]
            attn = softmax over n              (B=16, N=20000, D=256, H=8)

Sharding: batch dim B=16 split across 8 cores (2 batches/core), weights
replicated.

Per-core design (v2): avoid the PE-transpose-per-128x128-block pipeline
(which saturated the tensor engine at ~300us) by
  1. p-major loads: ld[p, s, d] = key[n0 + p*S + s, d] -> each of the 128
     partition lines is S*1KB contiguous in HBM (16KB DMA descriptors,
     near-line-rate ~358 GB/s).
  2. One DVE stream-transpose per load ([128, S*256], 32x32 blocks):
     ldt[32a+v, (s,b,u)] = key[n0+(32a+u)*S+s, 32b+v] -- d%32 moves onto
     partitions.
  3. 8 accumulating f32r matmuls with block-diagonal delta-weights
     lhsT_b[(a,v), 8a'+h] = (a==a') * Wk[h, 32b+v] compute scores for all
     128*S rows into ONE [32, S*32] PSUM bank (output partition = 8a'+h).
  4. One ScalarE exp per load (bias = q-term+b), writing prob with a
     strided AP so each partition holds a contiguous n-run; accum_out
     gives per-load softmax partial sums.
  5. Softmax normalization via tiny reduction matmuls + ACT copy-scale;
     contiguous output DMA per a'-group.
"""

import sys

import numpy as np

for _p in ("/opt/trn_rl_repo",):
    if _p not in sys.path:
        sys.path.append(_p)

from contextlib import ExitStack

import concourse.bass as bass
import concourse.bacc as bacc
import concourse.tile as tile
from concourse import mybir
from concourse.masks import make_identity

B, N, D, H = 16, 20000, 256, 8
NCORES = 8
BPC = B // NCORES  # batches per core
P = 128
F32 = mybir.dt.float32
F32R = mybir.dt.float32r
BF16 = mybir.dt.bfloat16
I32 = mybir.dt.int32

# p-major load plan: 9 loads of [128, 16] rows + 1 of [128, 12] + 32-row tail
LOADS = [16] * 9 + [12]  # S per load; rows = 128*S
MAIN_ROWS = sum(128 * s for s in LOADS)  # 19968
TAIL_ROWS = N - MAIN_ROWS  # 32


def _r(ap):
    return ap.bitcast(F32R)


def build_kernel(n=N, bpc=BPC):
    nc = bacc.Bacc("TRN2", target_bir_lowering=False, debug=False)
    q_in = nc.declare_dram_parameter("q", [bpc, D], F32, isOutput=False)
    k_in = nc.declare_dram_parameter("k", [bpc, n, D], F32, isOutput=False)
    w_in = nc.declare_dram_parameter("w", [H, 2 * D], F32, isOutput=False)
    b_in = nc.declare_dram_parameter("b", [H], F32, isOutput=False)
    out = nc.declare_dram_parameter("out", [bpc, H, n], F32, isOutput=True)

    ncols_main = sum(32 * s for s in LOADS)  # per-partition prob cols (4992)

    with ExitStack() as ctx:
        tc = ctx.enter_context(tile.TileContext(nc))
        consts = ctx.enter_context(tc.tile_pool(name="consts", bufs=1))
        loads = ctx.enter_context(tc.tile_pool(name="loads", bufs=3))
        ldts = ctx.enter_context(tc.tile_pool(name="ldts", bufs=3))
        probp = ctx.enter_context(tc.tile_pool(name="prob", bufs=2))
        small = ctx.enter_context(tc.tile_pool(name="small", bufs=2))
        psum_sc = ctx.enter_context(tc.tile_pool(name="psum_sc", bufs=3, space="PSUM"))
        psum_mi = ctx.enter_context(tc.tile_pool(name="psum_mi", bufs=2, space="PSUM"))

        identity = consts.tile([P, P], F32)
        make_identity(nc, identity)

        # --- constants ------------------------------------------------------
        w_sb = consts.tile([H, 2 * D], F32)
        nc.sync.dma_start(out=w_sb[:, :], in_=w_in[:, :])
        b_sb = consts.tile([H, 1], F32)
        nc.sync.dma_start(out=b_sb[:, :], in_=b_in[:])
        q_sb = consts.tile([1, bpc, D], F32)
        nc.sync.dma_start(out=q_sb[:, :, :], in_=q_in[:, :])

        # wqT[:, c, :]: Wq halves transposed to [d, h]
        wqT = consts.tile([P, 2, H], F32)
        for c in range(2):
            pt = psum_mi.tile([P, H], F32, tag="mi")
            nc.tensor.transpose(pt[:, :], w_sb[:, c * P:(c + 1) * P], identity[:H, :H])
            nc.vector.tensor_copy(out=wqT[:, c, :], in_=pt[:, :])

        # delta[(a,vp), j=(b32,e), 8a+h] = Wk[h, 64*b32 + 2*vp + e]
        # (the int32-pair stream transpose leaves d%2 in the free dim)
        delta32 = consts.tile([P, 8, 32], F32)
        nc.vector.memset(delta32[:, :, :], 0.0)
        wkT_small = consts.tile([32, 8, H], BF16)
        for j in range(8):
            b32, e = j // 2, j % 2
            pt = psum_mi.tile([32, H], F32, tag="mi")
            wk_slice = w_sb[:, D + 64 * b32:D + 64 * (b32 + 1)].rearrange(
                "h (vp e) -> h e vp", e=2
            )[:, e, :]
            nc.tensor.transpose(pt[:, :], wk_slice, identity[:H, :H])
            nc.vector.tensor_copy(out=wkT_small[:, j, :], in_=pt[:, :])
            for a in range(4):
                nc.vector.tensor_copy(
                    out=delta32[32 * a:32 * (a + 1), j, 8 * a:8 * (a + 1)],
                    in_=pt[:, :],
                )
        delta = consts.tile([P, 8, 32], BF16)
        nc.vector.tensor_copy(out=delta[:, :, :], in_=delta32[:, :, :])

        # E[h', 8a+h] = (h'==h): [I8 I8 I8 I8]  (f32: feeds tiny f32 matmuls)
        E = consts.tile([H, 32], F32)
        for a in range(4):
            nc.vector.tensor_copy(out=E[:, 8 * a:8 * (a + 1)], in_=identity[:H, :H])
        # F[(8a+h), h'] = (h==h'): 4 stacked I8 blocks, via transpose of E
        F = consts.tile([32, H], F32)
        ptF = psum_mi.tile([32, H], F32, tag="mi")
        nc.tensor.transpose(ptF[:, :], E[:, :], identity[:H, :H])
        nc.vector.tensor_copy(out=F[:, :], in_=ptF[:, :])

        # qT: query transposed to [d, i, c]
        qT = consts.tile([P, bpc, 2], F32)
        for i in range(bpc):
            for c in range(2):
                pt = psum_mi.tile([P, 1], F32, tag="mi")
                nc.tensor.transpose(
                    pt[:, :], q_sb[0:1, i, c * P:(c + 1) * P], identity[:1, :1]
                )
                nc.vector.tensor_copy(out=qT[:, i, c:c + 1], in_=pt[:, :])

        # qb8[h, i] = Wq @ q_i + bias
        qb8 = consts.tile([H, bpc], F32)
        for i in range(bpc):
            qp = psum_mi.tile([H, 1], F32, tag="mi")
            nc.tensor.matmul(
                qp[:, :], wqT[:, 0, :], qT[:, i, 0:1], start=True, stop=False
            )
            nc.tensor.matmul(
                qp[:, :], wqT[:, 1, :], qT[:, i, 1:2], start=False, stop=True
            )
            nc.vector.tensor_add(qb8[:, i:i + 1], qp[:, :], b_sb[:, :])
        # qb32[8a+h, i] = qb8[h, i]
        qb32 = consts.tile([32, bpc], F32)
        qp32 = psum_mi.tile([32, bpc], F32, tag="mi")
        nc.tensor.matmul(qp32[:, :], E[:, :], qb8[:, :], start=True, stop=True)
        nc.vector.tensor_copy(out=qb32[:, :], in_=qp32[:, :])

        # --- main loop ------------------------------------------------------
        for i in range(bpc):
            prob = probp.tile([32, ncols_main], F32, tag="prob")
            prob_t = probp.tile([H, TAIL_ROWS], F32, tag="probt")
            sums = small.tile([32, len(LOADS)], F32, tag="sums")
            sums_t = small.tile([H, 1], F32, tag="sumst")

            n0 = 0
            c0 = 0
            for L, S in enumerate(LOADS):
                rows = 128 * S
                ld = loads.tile([P, S, D], BF16, tag="load")
                nc.gpsimd.dma_start(
                    out=ld[:, :, :],
                    in_=k_in[i, n0:n0 + rows, :].rearrange("(p s) d -> p s d", s=S),
                )
                ldt = ldts.tile([P, S, 4, 32, 2], BF16, tag="ldt")
                nc.vector.transpose(
                    out=ldt[:, :, :, :, :].rearrange(
                        "p s b u e -> p (s b u e)"
                    ).bitcast(I32),
                    in_=ld[:, :, :].rearrange("p s d -> p (s d)").bitcast(I32),
                )
                scp = psum_sc.tile([32, S, 32], F32, tag="sc")
                for j in range(8):
                    b32, e = j // 2, j % 2
                    nc.tensor.matmul(
                        scp[:, :, :],
                        delta[:, j, :],
                        ldt[:, :, b32, :, e],
                        start=(j == 0),
                        stop=(j == 7),
                    )
                # prob cols: c0 + u*S + s  (natural n order per partition)
                nc.scalar.activation(
                    out=prob[:, c0:c0 + 32 * S].rearrange("p (u s) -> p s u", s=S),
                    in_=scp[:, :, :],
                    func=mybir.ActivationFunctionType.Exp,
                    bias=qb32[:, i:i + 1],
                    scale=1.0,
                    accum_out=sums[:, L:L + 1],
                )
                n0 += rows
                c0 += 32 * S

            # 32-row tail: natural layout, K=32 matmuls on partitions 0-7
            ld_tb = loads.tile([TAIL_ROWS, D], BF16, tag="loadtb")
            nc.gpsimd.dma_start(out=ld_tb[:, :], in_=k_in[i, MAIN_ROWS:n, :])
            ldt_tb = ldts.tile([TAIL_ROWS, 4, 32, 2], BF16, tag="ldttb")
            nc.vector.transpose(
                out=ldt_tb[:, :, :, :].rearrange("p b u e -> p (b u e)").bitcast(I32),
                in_=ld_tb[:, :].bitcast(I32),
            )
            sct = psum_sc.tile([H, TAIL_ROWS], F32, tag="sct")
            for j in range(8):
                b32, e = j // 2, j % 2
                nc.tensor.matmul(
                    sct[:, :],
                    wkT_small[:, j, :],
                    ldt_tb[:, b32, :, e],
                    start=(j == 0),
                    stop=(j == 7),
                )
            nc.scalar.activation(
                out=prob_t[:, :],
                in_=sct[:, :],
                func=mybir.ActivationFunctionType.Exp,
                bias=qb8[:, i:i + 1],
                scale=1.0,
                accum_out=sums_t[:, :],
            )

            # totals: tot8[h] = sum_a sums[(8a+h), :] + sums_t[h]  (all-f32 tinies)
            sums_r = small.tile([32, 1], F32, tag="sumsr")
            nc.vector.reduce_sum(
                out=sums_r[:, :], in_=sums[:, :], axis=mybir.AxisListType.X
            )
            tot8 = psum_mi.tile([H, 1], F32, tag="mi")
            nc.tensor.matmul(
                tot8[:, :], F[:, :], sums_r[:, :], start=True, stop=False
            )
            nc.tensor.matmul(
                tot8[:, :], identity[:H, :H], sums_t[:, :], start=False, stop=True
            )
            rec8 = small.tile([H, 1], F32, tag="rec8")
            nc.vector.reciprocal(out=rec8[:, :], in_=tot8[:, :])
            rec32p = psum_mi.tile([32, 1], F32, tag="mi")
            nc.tensor.matmul(
                rec32p[:, :], E[:, :], rec8[:, :], start=True, stop=True
            )
            rec32 = small.tile([32, 1], F32, tag="rec32")
            nc.vector.tensor_copy(out=rec32[:, :], in_=rec32p[:, :])

            # normalize
            nc.scalar.activation(
                out=prob[:, :],
                in_=prob[:, :],
                func=mybir.ActivationFunctionType.Copy,
                bias=0.0,
                scale=rec32[:, :],
            )
            nc.scalar.activation(
                out=prob_t[:, :],
                in_=prob_t[:, :],
                func=mybir.ActivationFunctionType.Copy,
                bias=0.0,
                scale=rec8[:, :],
            )

            # output: per a'-group, n = n_base(L) + a*32*S + (u*S+s)
            full = [(L, S) for L, S in enumerate(LOADS) if S == 16]
            nfull = len(full)  # 9
            for a in range(4):
                # loads 0..8 (S=16): n = L*2048 + a*512 + c
                nc.sync.dma_start(
                    out=out[i, :, 0:nfull * 2048].rearrange(
                        "h (l x) -> h l x", x=2048
                    )[:, :, a * 512:(a + 1) * 512],
                    in_=prob[8 * a:8 * (a + 1), 0:nfull * 512].rearrange(
                        "h (l c) -> h l c", c=512
                    ),
                )
                # load 9 (S=12): n = 18432 + a*384 + c
                nc.sync.dma_start(
                    out=out[i, :, nfull * 2048 + a * 384:nfull * 2048 + (a + 1) * 384],
                    in_=prob[8 * a:8 * (a + 1), nfull * 512:nfull * 512 + 384],
                )
            nc.sync.dma_start(out=out[i, :, MAIN_ROWS:n], in_=prob_t[:, :])

    nc.compile()
    return nc


_NC_CACHE = {}


def _get_nc():
    if "nc" not in _NC_CACHE:
        _NC_CACHE["nc"] = build_kernel()
    return _NC_CACHE["nc"]


def kernel(query, key, W, b):
    from concourse.bass_utils import run_bass_kernel_spmd

    query = np.ascontiguousarray(np.asarray(query, np.float32).reshape(B, D))
    key = np.ascontiguousarray(np.asarray(key, np.float32))
    W = np.ascontiguousarray(np.asarray(W, np.float32))
    b = np.ascontiguousarray(np.asarray(b, np.float32))

    nc = _get_nc()
    in_maps = []
    for c in range(NCORES):
        s = slice(BPC * c, BPC * (c + 1))
        in_maps.append(
            {
                "q": query[s],
                "k": key[s],
                "w": W,
                "b": b,
            }
        )
    res = run_bass_kernel_spmd(nc, in_maps, list(range(NCORES))).results
    return np.concatenate([res[c]["out"] for c in range(NCORES)], axis=0)


# revision 21
# speedup vs baseline: 1.4848x; 1.0538x over previous
"""Trainium2 Bass kernel: fused concat-linear attention map + softmax.

reference:  scores[b,h,n] = key[b,n,:]@Wk[h,:] + query[b,0,:]@Wq[h,:] + bias[h]
            attn = softmax over n              (B=16, N=20000, D=256, H=8)

Sharding: batch dim B=16 split across 8 cores (2 batches/core), weights
replicated.

Per-core design (v2): avoid the PE-transpose-per-128x128-block pipeline
(which saturated the tensor engine at ~300us) by
  1. p-major loads: ld[p, s, d] = key[n0 + p*S + s, d] -> each of the 128
     partition lines is S*1KB contiguous in HBM (16KB DMA descriptors,
     near-line-rate ~358 GB/s).
  2. One DVE stream-transpose per load ([128, S*256], 32x32 blocks):
     ldt[32a+v, (s,b,u)] = key[n0+(32a+u)*S+s, 32b+v] -- d%32 moves onto
     partitions.
  3. 8 accumulating f32r matmuls with block-diagonal delta-weights
     lhsT_b[(a,v), 8a'+h] = (a==a') * Wk[h, 32b+v] compute scores for all
     128*S rows into ONE [32, S*32] PSUM bank (output partition = 8a'+h).
  4. One ScalarE exp per load (bias = q-term+b), writing prob with a
     strided AP so each partition holds a contiguous n-run; accum_out
     gives per-load softmax partial sums.
  5. Softmax normalization via tiny reduction matmuls + ACT copy-scale;
     contiguous output DMA per a'-group.
"""

import sys

import numpy as np

for _p in ("/opt/trn_rl_repo",):
    if _p not in sys.path:
        sys.path.append(_p)

from contextlib import ExitStack

import concourse.bass as bass
import concourse.bacc as bacc
import concourse.tile as tile
from concourse import mybir
from concourse.masks import make_identity

B, N, D, H = 16, 20000, 256, 8
NCORES = 8
BPC = B // NCORES  # batches per core
P = 128
F32 = mybir.dt.float32
F32R = mybir.dt.float32r
BF16 = mybir.dt.bfloat16
I32 = mybir.dt.int32

# p-major load plan: 9 loads of [128, 16] rows + 1 of [128, 12] + 32-row tail
LOADS = [16] * 9 + [12]  # S per load; rows = 128*S
MAIN_ROWS = sum(128 * s for s in LOADS)  # 19968
TAIL_ROWS = N - MAIN_ROWS  # 32


def _r(ap):
    return ap.bitcast(F32R)


def build_kernel(n=N, bpc=BPC):
    nc = bacc.Bacc("TRN2", target_bir_lowering=False, debug=False)
    q_in = nc.declare_dram_parameter("q", [bpc, D], F32, isOutput=False)
    k_in = nc.declare_dram_parameter("k", [bpc, n, D], F32, isOutput=False)
    w_in = nc.declare_dram_parameter("w", [H, 2 * D], F32, isOutput=False)
    b_in = nc.declare_dram_parameter("b", [H], F32, isOutput=False)
    out = nc.declare_dram_parameter("out", [bpc, H, n], F32, isOutput=True)

    ncols_main = sum(32 * s for s in LOADS)  # per-partition prob cols (4992)

    with ExitStack() as ctx:
        tc = ctx.enter_context(tile.TileContext(nc))
        consts = ctx.enter_context(tc.tile_pool(name="consts", bufs=1))
        loads = ctx.enter_context(tc.tile_pool(name="loads", bufs=4))
        ldts = ctx.enter_context(tc.tile_pool(name="ldts", bufs=3))
        probp = ctx.enter_context(tc.tile_pool(name="prob", bufs=2))
        small = ctx.enter_context(tc.tile_pool(name="small", bufs=2))
        psum_sc = ctx.enter_context(tc.tile_pool(name="psum_sc", bufs=3, space="PSUM"))
        psum_mi = ctx.enter_context(tc.tile_pool(name="psum_mi", bufs=2, space="PSUM"))

        identity = consts.tile([P, P], F32)
        make_identity(nc, identity)

        # --- constants ------------------------------------------------------
        w_sb = consts.tile([H, 2 * D], F32)
        nc.sync.dma_start(out=w_sb[:, :], in_=w_in[:, :])
        b_sb = consts.tile([H, 1], F32)
        nc.sync.dma_start(out=b_sb[:, :], in_=b_in[:])
        q_sb = consts.tile([1, bpc, D], F32)
        nc.sync.dma_start(out=q_sb[:, :, :], in_=q_in[:, :])

        # wqT[:, c, :]: Wq halves transposed to [d, h]
        wqT = consts.tile([P, 2, H], F32)
        for c in range(2):
            pt = psum_mi.tile([P, H], F32, tag="mi")
            nc.tensor.transpose(pt[:, :], w_sb[:, c * P:(c + 1) * P], identity[:H, :H])
            nc.vector.tensor_copy(out=wqT[:, c, :], in_=pt[:, :])

        # delta[(a,vp), j=(b32,e), 8a+h] = Wk[h, 64*b32 + 2*vp + e]
        # (the int32-pair stream transpose leaves d%2 in the free dim)
        delta32 = consts.tile([P, 8, 32], F32)
        nc.vector.memset(delta32[:, :, :], 0.0)
        wkT_small = consts.tile([32, 8, H], BF16)
        for j in range(8):
            b32, e = j // 2, j % 2
            pt = psum_mi.tile([32, H], F32, tag="mi")
            wk_slice = w_sb[:, D + 64 * b32:D + 64 * (b32 + 1)].rearrange(
                "h (vp e) -> h e vp", e=2
            )[:, e, :]
            nc.tensor.transpose(pt[:, :], wk_slice, identity[:H, :H])
            nc.vector.tensor_copy(out=wkT_small[:, j, :], in_=pt[:, :])
            for a in range(4):
                nc.vector.tensor_copy(
                    out=delta32[32 * a:32 * (a + 1), j, 8 * a:8 * (a + 1)],
                    in_=pt[:, :],
                )
        delta = consts.tile([P, 8, 32], BF16)
        nc.vector.tensor_copy(out=delta[:, :, :], in_=delta32[:, :, :])

        # E[h', 8a+h] = (h'==h): [I8 I8 I8 I8]  (f32: feeds tiny f32 matmuls)
        E = consts.tile([H, 32], F32)
        for a in range(4):
            nc.vector.tensor_copy(out=E[:, 8 * a:8 * (a + 1)], in_=identity[:H, :H])
        # F[(8a+h), h'] = (h==h'): 4 stacked I8 blocks, via transpose of E
        F = consts.tile([32, H], F32)
        ptF = psum_mi.tile([32, H], F32, tag="mi")
        nc.tensor.transpose(ptF[:, :], E[:, :], identity[:H, :H])
        nc.vector.tensor_copy(out=F[:, :], in_=ptF[:, :])

        # qT: query transposed to [d, i, c]
        qT = consts.tile([P, bpc, 2], F32)
        for i in range(bpc):
            for c in range(2):
                pt = psum_mi.tile([P, 1], F32, tag="mi")
                nc.tensor.transpose(
                    pt[:, :], q_sb[0:1, i, c * P:(c + 1) * P], identity[:1, :1]
                )
                nc.vector.tensor_copy(out=qT[:, i, c:c + 1], in_=pt[:, :])

        # qb8[h, i] = Wq @ q_i + bias
        qb8 = consts.tile([H, bpc], F32)
        for i in range(bpc):
            qp = psum_mi.tile([H, 1], F32, tag="mi")
            nc.tensor.matmul(
                qp[:, :], wqT[:, 0, :], qT[:, i, 0:1], start=True, stop=False
            )
            nc.tensor.matmul(
                qp[:, :], wqT[:, 1, :], qT[:, i, 1:2], start=False, stop=True
            )
            nc.vector.tensor_add(qb8[:, i:i + 1], qp[:, :], b_sb[:, :])
        # qb32[8a+h, i] = qb8[h, i]
        qb32 = consts.tile([32, bpc], F32)
        qp32 = psum_mi.tile([32, bpc], F32, tag="mi")
        nc.tensor.matmul(qp32[:, :], E[:, :], qb8[:, :], start=True, stop=True)
        nc.vector.tensor_copy(out=qb32[:, :], in_=qp32[:, :])

        # --- main loop ------------------------------------------------------
        for i in range(bpc):
            prob = probp.tile([32, ncols_main], F32, tag="prob")
            prob_t = probp.tile([H, TAIL_ROWS], F32, tag="probt")
            sums = small.tile([32, len(LOADS)], F32, tag="sums")
            sums_t = small.tile([H, 1], F32, tag="sumst")

            n0 = 0
            c0 = 0
            for L, S in enumerate(LOADS):
                rows = 128 * S
                ld = loads.tile([P, S, D], BF16, tag="load")
                nc.gpsimd.dma_start(
                    out=ld[:, :, :],
                    in_=k_in[i, n0:n0 + rows, :].rearrange("(p s) d -> p s d", s=S),
                )
                ldt = ldts.tile([P, S, 4, 32, 2], BF16, tag="ldt")
                nc.vector.transpose(
                    out=ldt[:, :, :, :, :].rearrange(
                        "p s b u e -> p (s b u e)"
                    ).bitcast(I32),
                    in_=ld[:, :, :].rearrange("p s d -> p (s d)").bitcast(I32),
                )
                scp = psum_sc.tile([32, S, 32], F32, tag="sc")
                for j in range(8):
                    b32, e = j // 2, j % 2
                    nc.tensor.matmul(
                        scp[:, :, :],
                        delta[:, j, :],
                        ldt[:, :, b32, :, e],
                        start=(j == 0),
                        stop=(j == 7),
                    )
                # prob cols: c0 + u*S + s  (natural n order per partition)
                nc.scalar.activation(
                    out=prob[:, c0:c0 + 32 * S].rearrange("p (u s) -> p s u", s=S),
                    in_=scp[:, :, :],
                    func=mybir.ActivationFunctionType.Exp,
                    bias=qb32[:, i:i + 1],
                    scale=1.0,
                    accum_out=sums[:, L:L + 1],
                )
                n0 += rows
                c0 += 32 * S

            # 32-row tail: natural layout, K=32 matmuls on partitions 0-7
            ld_tb = loads.tile([TAIL_ROWS, D], BF16, tag="loadtb")
            nc.gpsimd.dma_start(out=ld_tb[:, :], in_=k_in[i, MAIN_ROWS:n, :])
            ldt_tb = ldts.tile([TAIL_ROWS, 4, 32, 2], BF16, tag="ldttb")
            nc.vector.transpose(
                out=ldt_tb[:, :, :, :].rearrange("p b u e -> p (b u e)").bitcast(I32),
                in_=ld_tb[:, :].bitcast(I32),
            )
            sct = psum_sc.tile([H, TAIL_ROWS], F32, tag="sct")
            for j in range(8):
                b32, e = j // 2, j % 2
                nc.tensor.matmul(
                    sct[:, :],
                    wkT_small[:, j, :],
                    ldt_tb[:, b32, :, e],
                    start=(j == 0),
                    stop=(j == 7),
                )
            nc.scalar.activation(
                out=prob_t[:, :],
                in_=sct[:, :],
                func=mybir.ActivationFunctionType.Exp,
                bias=qb8[:, i:i + 1],
                scale=1.0,
                accum_out=sums_t[:, :],
            )

            # totals: tot8[h] = sum_a sums[(8a+h), :] + sums_t[h]  (all-f32 tinies)
            sums_r = small.tile([32, 1], F32, tag="sumsr")
            nc.vector.reduce_sum(
                out=sums_r[:, :], in_=sums[:, :], axis=mybir.AxisListType.X
            )
            tot8 = psum_mi.tile([H, 1], F32, tag="mi")
            nc.tensor.matmul(
                tot8[:, :], F[:, :], sums_r[:, :], start=True, stop=False
            )
            nc.tensor.matmul(
                tot8[:, :], identity[:H, :H], sums_t[:, :], start=False, stop=True
            )
            rec8 = small.tile([H, 1], F32, tag="rec8")
            nc.vector.reciprocal(out=rec8[:, :], in_=tot8[:, :])
            rec32p = psum_mi.tile([32, 1], F32, tag="mi")
            nc.tensor.matmul(
                rec32p[:, :], E[:, :], rec8[:, :], start=True, stop=True
            )
            rec32 = small.tile([32, 1], F32, tag="rec32")
            nc.vector.tensor_copy(out=rec32[:, :], in_=rec32p[:, :])

            # normalize + output, chunked so scale/DMA pipeline at batch end.
            # chunk 0: loads 0..4 (cols 0..2560), chunk 1: loads 5..8
            # (cols 2560..4608), then the S=12 load + tails.
            nfull = 9
            nc.vector.tensor_scalar_mul(
                prob[:, 0:5 * 512], prob[:, 0:5 * 512], rec32[:, :]
            )
            for a in range(4):
                nc.sync.dma_start(
                    out=out[i, :, 0:5 * 2048].rearrange("h (l x) -> h l x", x=2048)[
                        :, :, a * 512:(a + 1) * 512
                    ],
                    in_=prob[8 * a:8 * (a + 1), 0:5 * 512].rearrange(
                        "h (l c) -> h l c", c=512
                    ),
                )
            nc.vector.tensor_scalar_mul(
                prob[:, 5 * 512:nfull * 512],
                prob[:, 5 * 512:nfull * 512],
                rec32[:, :],
            )
            for a in range(4):
                nc.sync.dma_start(
                    out=out[i, :, 5 * 2048:nfull * 2048].rearrange(
                        "h (l x) -> h l x", x=2048
                    )[:, :, a * 512:(a + 1) * 512],
                    in_=prob[8 * a:8 * (a + 1), 5 * 512:nfull * 512].rearrange(
                        "h (l c) -> h l c", c=512
                    ),
                )
            nc.vector.tensor_scalar_mul(
                prob[:, nfull * 512:], prob[:, nfull * 512:], rec32[:, :]
            )
            nc.vector.tensor_scalar_mul(prob_t[:, :], prob_t[:, :], rec8[:, :])
            for a in range(4):
                nc.sync.dma_start(
                    out=out[i, :, nfull * 2048 + a * 384:nfull * 2048 + (a + 1) * 384],
                    in_=prob[8 * a:8 * (a + 1), nfull * 512:nfull * 512 + 384],
                )
            nc.sync.dma_start(out=out[i, :, MAIN_ROWS:n], in_=prob_t[:, :])

    nc.compile()
    return nc


_NC_CACHE = {}


def _get_nc():
    if "nc" not in _NC_CACHE:
        _NC_CACHE["nc"] = build_kernel()
    return _NC_CACHE["nc"]


def kernel(query, key, W, b):
    from concourse.bass_utils import run_bass_kernel_spmd

    query = np.ascontiguousarray(np.asarray(query, np.float32).reshape(B, D))
    key = np.ascontiguousarray(np.asarray(key, np.float32))
    W = np.ascontiguousarray(np.asarray(W, np.float32))
    b = np.ascontiguousarray(np.asarray(b, np.float32))

    nc = _get_nc()
    in_maps = []
    for c in range(NCORES):
        s = slice(BPC * c, BPC * (c + 1))
        in_maps.append(
            {
                "q": query[s],
                "k": key[s],
                "w": W,
                "b": b,
            }
        )
    res = run_bass_kernel_spmd(nc, in_maps, list(range(NCORES))).results
    return np.concatenate([res[c]["out"] for c in range(NCORES)], axis=0)
